# revision 25
# baseline (speedup 1.0000x reference)
"""Trainium2 Bass kernel for nn_CrossAttentionLayer (m=n=1024, d=2048).

Math: f = relu(term1 + term23 + term4 + ffn_b), with W1..W4 = ffn_w.reshape(n,4,d):
  term1  = sum_i u_p[i] . W1[i]
  term23 = sum_i [sum_k E[i,k] M2[i,k]] / [sum_k E[i,k]]   (row softmax)
  term4  = sum_k [sum_i E[i,k] M4[i,k]] / [sum_i E[i,k]]   (col softmax)
where E = exp(S - 6),  S[i,k] = u_p[i].w1 + u_c[k].w2 + (u_p[i]*w3).u_c[k],
  M2 = (W2 + u_p*W3) @ u_c.T = C @ u_c.T,   M4 = u_p @ (u_c*W4).T = u_p @ V4.T.
Because softmax ratios cancel any per-row / per-column / global shift, ONE
exp(S) array serves both softmaxes; the global -6 keeps exp in fp16 range.

Sharding: 8 cores = 4 mention shards (I, 256 rows) x 2 candidate shards
(K, 512 cols).  Each core computes its [256,512] blocks of S, M2, M4 (three
fp16 matmuls, d=2048 contraction in 16 psum-accumulated chunks), then
  Z_part[i] = sum_k E,  G_part[i] = sum_k E*M2   (free-axis accumulators)
  Z'_part[k] = sum_i E, N_part[k] = sum_i E*M4   (ones-vector matmuls)
term1 is split over the d-chunks between the 2 K-cores of each I shard.
The host fuses C and V4 into single fp16 operands during packing, adds the
row bias r_i = u_p[i].w1 - 6 via the exp activation's per-partition bias,
sums the tiny per-core partials in float64, and applies bias + relu.

Layout: every matmul operand is pre-transposed ([d, rows]) then packed
p-major ([128, chunk, rows]) so each DMA is contiguous per partition; DMAs
are issued in 2-chunk groups interleaved across the sync/scalar HWDGE
queues in consumption order so the PE can start within ~2us and stay dense
(HAM stays warm).  The matmul loop is chunk-outer over 6 persistent PSUM
accumulators (T/M2/M4 x 2 row tiles) so each 384KB of arriving DMA unlocks
~1.3us of matmul work.
"""

import sys

sys.path.insert(0, "/opt/trn_rl_repo")

import numpy as np

import concourse.bass as bass
import concourse.tile as tile
from concourse import mybir
from concourse.bass_utils import run_bass_kernel_spmd

F32 = mybir.dt.float32
F16 = mybir.dt.float16

M = 1024  # mentions
N = 1024  # candidates
D = 2048  # feature dim (contraction)
NCORES = 8
ISH = 256  # mention rows per core
KSH = 512  # candidate cols per core
NI = M // ISH  # 4 mention shards
NK = N // KSH  # 2 candidate shards
CH = D // 128  # 16 contraction chunks
ITILES = ISH // 128  # 2
TCH = CH // NK  # 8 term1 chunks per core
GC = 2  # chunks per input DMA group

# ---------------------------------------------------------------------------
# Workaround: the pinned neuronxcc walrus accepts fewer sync waits per
# instruction than Tile's semaphore assignment attaches.  After scheduling,
# hoist excess waits of any over-capacity instruction onto same-engine
# EventSemaphores inserted right before it; each engine executes its stream
# in order, so the waits still gate the instruction.
_DEFAULT_CAP = 1
_wfix_counter = [0]


def _legalize_waits(nc: bass.Bass) -> None:
    for f in nc.m.functions:
        for bb in f.blocks:
            il = bb.instructions
            out = []
            for inst in il:
                si = inst.sync_info
                waits = list(si.on_wait) if si and si.on_wait else []
                if len(waits) > _DEFAULT_CAP:
                    keep = waits[:_DEFAULT_CAP]
                    for w in waits[_DEFAULT_CAP:]:
                        _wfix_counter[0] += 1
                        out.append(
                            mybir.InstEventSemaphore(
                                name=f"I-wfix-{_wfix_counter[0]}",
                                engine=inst.engine,
                                ins=[],
                                outs=[],
                                sync_info=mybir.SyncInfo(on_wait=[w], on_update=[]),
                            )
                        )
                    inst.sync_info = mybir.SyncInfo(
                        on_wait=keep, on_update=list(si.on_update or [])
                    )
                out.append(inst)
            bb.instructions = out


# ---------------------------------------------------------------------------
def _emit(nc: bass.Bass, tc: tile.TileContext, io: dict) -> None:
    mult = mybir.AluOpType.mult
    add = mybir.AluOpType.add

    upt_r = io["upt"].ap().rearrange("p (c i) -> p c i", c=CH)
    cmt_r = io["cmt"].ap().rearrange("p (c i) -> p c i", c=CH)
    uct_r = io["uct"].ap().rearrange("p (c k) -> p c k", c=CH)
    v4t_r = io["v4t"].ap().rearrange("p (c k) -> p c k", c=CH)
    w1p_r = io["w1p"].ap().rearrange("p (c i) -> p c i", c=TCH)
    wvr_r = io["wvr"].ap()

    import contextlib

    ctx = contextlib.ExitStack()
    singles = ctx.enter_context(tc.tile_pool(name="singles", bufs=1))
    scratch = ctx.enter_context(tc.tile_pool(name="scratch", bufs=2))
    psum = ctx.enter_context(tc.tile_pool(name="psum", bufs=1, space="PSUM"))

    upt = singles.tile([128, CH, ISH], F16)
    asp = singles.tile([128, CH, ISH], F16)
    cmt = singles.tile([128, CH, ISH], F16)
    uct = singles.tile([128, CH, KSH], F16)
    v4t = singles.tile([128, CH, KSH], F16)
    w1p = singles.tile([128, TCH, ISH], F16)
    wvr = singles.tile([128, CH * 2 + ITILES], F32)
    wv = wvr[:, : CH * 2].rearrange("p (c v) -> p c v", c=CH)
    rb = wvr[:, CH * 2 :]
    ones = singles.tile([128, 1], F16)

    ev = [singles.tile([128, KSH], F16, name=f"ev{it}", tag=f"ev{it}") for it in range(ITILES)]
    zg = singles.tile([128, 2 * ITILES], F32)  # cols: Z it0, Z it1, G it0, G it1
    ae = singles.tile([128, TCH], F32)
    nzs = singles.tile([1, 2 * KSH], F32)

    # PSUM: 6 full-kernel accumulators + 2 column-sum banks = all 8 banks.
    tps = [psum.tile([128, KSH], F32, name=f"tps{it}", tag=f"tps{it}") for it in range(ITILES)]
    mps = [psum.tile([128, KSH], F32, name=f"mps{it}", tag=f"mps{it}") for it in range(ITILES)]
    qps = [psum.tile([128, KSH], F32, name=f"qps{it}", tag=f"qps{it}") for it in range(ITILES)]
    zcp = psum.tile([1, KSH], F32)
    ncp = psum.tile([1, KSH], F32)

    # Warm-up operands first so the dummy matmuls can start right after the
    # prologue (~6.5us) and have the HAM activity window at full 2.4GHz by
    # the time real data lands (~11us).
    dum = singles.tile([128, KSH], F16)
    nc.gpsimd.memset(ones, 1.0)
    nc.gpsimd.memset(dum, 0.0)
    for _ in range(14):
        nc.tensor.matmul(zcp, lhsT=ones, rhs=dum, start=True, stop=True)

    # Input DMAs.  Aggregate input streaming is pinned at ~300GB/s and each
    # transfer has ~4us trigger-to-semaphore latency, so: two HWDGE queues,
    # byte-balanced, laddered into ~0.4MB groups interleaved in chunk-outer
    # consumption order so the PE tracks arriving data to the end.  Each
    # queue carries one full matmul chain: sync feeds T (upt->asp + uct),
    # scalar feeds M4 then M2 (v4t, cmt) to match the PE's per-chunk order.
    nc.sync.dma_start(out=wvr, in_=wvr_r)
    LADDER = [(0, 2), (2, 5), (5, 8), (8, 11), (11, 14), (14, 16)]
    for c0, c1 in LADDER:
        sl = slice(c0, c1)
        nc.sync.dma_start(out=upt[:, sl, :], in_=upt_r[:, sl, :])
        nc.sync.dma_start(out=v4t[:, sl, :], in_=v4t_r[:, sl, :])
        nc.scalar.dma_start(out=uct[:, sl, :], in_=uct_r[:, sl, :])
        nc.scalar.dma_start(out=cmt[:, sl, :], in_=cmt_r[:, sl, :])
    nc.scalar.dma_start(out=w1p, in_=w1p_r)

    # ASp = u_p*w3 + w2 (per-partition scalars), chunk by chunk as upt lands.
    for c in range(CH):
        nc.vector.tensor_scalar(
            out=asp[:, c, :],
            in0=upt[:, c, :],
            scalar1=wv[:, c, 0:1],
            scalar2=wv[:, c, 1:2],
            op0=mult,
            op1=add,
        )

    # term1 partials: ae[:, c] = rowsum(u_p * W1) over this core's d-chunks.
    # The host rotates every input's chunk axis by kk*TCH, so this core's
    # term1 chunks are upt chunks 0..TCH-1 on every core.
    for c in range(TCH):
        t1o = scratch.tile([128, ISH], F16, tag="t1o")
        nc.vector.scalar_tensor_tensor(
            out=t1o,
            in0=upt[:, c, :],
            scalar=1.0,
            in1=w1p[:, c, :],
            op0=mult,
            op1=mult,
            accum_out=ae[:, c : c + 1],
        )

    # Main contraction: chunk-outer so each arriving chunk group unlocks all
    # six accumulating matmuls for that chunk.
    for c in range(CH):
        st = c == 0
        sp = c == CH - 1
        for it in range(ITILES):
            lhs = asp[:, c, it * 128 : (it + 1) * 128]
            nc.tensor.matmul(tps[it], lhsT=lhs, rhs=uct[:, c, :], start=st, stop=sp)
        for it in range(ITILES):
            lhs = upt[:, c, it * 128 : (it + 1) * 128]
            nc.tensor.matmul(qps[it], lhsT=lhs, rhs=v4t[:, c, :], start=st, stop=sp)
        for it in range(ITILES):
            lhs = cmt[:, c, it * 128 : (it + 1) * 128]
            nc.tensor.matmul(mps[it], lhsT=lhs, rhs=uct[:, c, :], start=st, stop=sp)

    # Tail: E = exp(T + r), row sums via accum, products, column sums.
    for it in range(ITILES):
        nc.scalar.activation(
            out=ev[it],
            in_=tps[it],
            func=mybir.ActivationFunctionType.Exp,
            bias=rb[:, it : it + 1],
            accum_out=zg[:, it : it + 1],
        )
        h2 = scratch.tile([128, KSH], F16, tag="h2")
        nc.vector.scalar_tensor_tensor(
            out=h2,
            in0=ev[it],
            scalar=1.0,
            in1=mps[it],
            op0=mult,
            op1=mult,
            accum_out=zg[:, ITILES + it : ITILES + it + 1],
        )
        h4 = scratch.tile([128, KSH], F16, tag="h4")
        nc.vector.scalar_tensor_tensor(
            out=h4,
            in0=ev[it],
            scalar=1.0,
            in1=qps[it],
            op0=mult,
            op1=mult,
        )
        nc.tensor.matmul(
            zcp, lhsT=ones, rhs=ev[it], start=(it == 0), stop=(it == ITILES - 1)
        )
        nc.tensor.matmul(
            ncp, lhsT=ones, rhs=h4, start=(it == 0), stop=(it == ITILES - 1)
        )

    nc.vector.tensor_copy(out=nzs[:, 0:KSH], in_=zcp)
    nc.vector.tensor_copy(out=nzs[:, KSH:], in_=ncp)

    nc.sync.dma_start(out=io["out_zg"].ap(), in_=zg)
    nc.gpsimd.dma_start(out=io["out_ae"].ap(), in_=ae)
    nc.scalar.dma_start(out=io["out_nz"].ap(), in_=nzs)
    ctx.close()


def _build() -> bass.Bass:
    nc = bass.Bass()
    io = {}
    io["upt"] = nc.declare_dram_parameter("upt", [128, CH * ISH], F16, isOutput=False)
    io["cmt"] = nc.declare_dram_parameter("cmt", [128, CH * ISH], F16, isOutput=False)
    io["uct"] = nc.declare_dram_parameter("uct", [128, CH * KSH], F16, isOutput=False)
    io["v4t"] = nc.declare_dram_parameter("v4t", [128, CH * KSH], F16, isOutput=False)
    io["w1p"] = nc.declare_dram_parameter("w1p", [128, TCH * ISH], F16, isOutput=False)
    io["wvr"] = nc.declare_dram_parameter(
        "wvr", [128, CH * 2 + ITILES], F32, isOutput=False
    )
    io["out_zg"] = nc.declare_dram_parameter(
        "out_zg", [128, 2 * ITILES], F32, isOutput=True
    )
    io["out_ae"] = nc.declare_dram_parameter("out_ae", [128, TCH], F32, isOutput=True)
    io["out_nz"] = nc.declare_dram_parameter(
        "out_nz", [1, 2 * KSH], F32, isOutput=True
    )
    with tile.TileContext(nc) as tc:
        _emit(nc, tc, io)
    _legalize_waits(nc)
    return nc


_NC_CACHE: bass.Bass | None = None


def _get_nc() -> bass.Bass:
    global _NC_CACHE
    if _NC_CACHE is None:
        _NC_CACHE = _build()
    return _NC_CACHE


def _pack(a2d: np.ndarray) -> np.ndarray:
    """[D, x] (d-major) -> [128, CH_x * x] p-major so each partition's data
    is one contiguous DRAM run per chunk group."""
    d, x = a2d.shape
    ch = d // 128
    return np.ascontiguousarray(
        a2d.reshape(ch, 128, x).transpose(1, 0, 2).reshape(128, ch * x)
    )


def _in_maps(u_p, u_c, w_a, ffn_w):
    f16 = np.float16
    W = ffn_w.reshape(N, 4, D)
    wa = w_a[0]
    w1, w2, w3 = wa[:D], wa[D : 2 * D], wa[2 * D :]

    C = W[:, 1, :] + u_p * W[:, 2, :]  # [m, d]
    V4 = u_c * W[:, 3, :]  # [n, d]
    r = u_p @ w1 - 6.0  # [m] row bias, shifted into fp16-exp range

    u_pT = _pack(u_p.T.astype(f16)).reshape(128, CH, M)
    u_cT = _pack(u_c.T.astype(f16)).reshape(128, CH, N)
    CT = _pack(C.T.astype(f16)).reshape(128, CH, M)
    V4T = _pack(V4.T.astype(f16)).reshape(128, CH, N)
    W1T = _pack(W[:, 0, :].T.astype(f16)).reshape(128, CH, M)
    wv = _pack(np.stack([w3, w2], axis=1).astype(np.float32)).reshape(128, CH, 2)

    def rolled(packed, sl_cols, kk, c0=0, c1=CH):
        # Rotate the chunk axis by kk*TCH: the contraction is order-
        # invariant, and it puts this core's term1 d-chunks at 0..TCH-1.
        v = np.roll(packed[:, :, sl_cols], -kk * TCH, axis=1)[:, c0:c1, :]
        return np.ascontiguousarray(v).reshape(128, -1)

    maps = []
    for ii in range(NI):
        isl = slice(ISH * ii, ISH * (ii + 1))
        rbl = np.ascontiguousarray(
            r[isl].astype(np.float32).reshape(ITILES, 128).T
        )
        for kk in range(NK):
            ksl = slice(KSH * kk, KSH * (kk + 1))
            wvk = rolled(wv, slice(0, 2), kk)
            maps.append(
                {
                    "upt": rolled(u_pT, isl, kk),
                    "cmt": rolled(CT, isl, kk),
                    "uct": rolled(u_cT, ksl, kk),
                    "v4t": rolled(V4T, ksl, kk),
                    "w1p": rolled(W1T, isl, kk, 0, TCH),
                    "wvr": np.ascontiguousarray(
                        np.concatenate([wvk, rbl], axis=1)
                    ),
                }
            )
    return maps


def _reduce(results: list[dict], ffn_b) -> float:
    """Pre-relu scalar from the per-core partial sums, in float64."""
    total = 0.0
    # term1: each core covers its I rows x its TCH d-chunks exactly once.
    for r_ in results:
        total += r_["out_ae"].sum(dtype=np.float64)
    # term23: per I shard, Z/G summed over the 2 K cores, then sum_i G/Z.
    for ii in range(NI):
        zg0 = results[ii * NK]["out_zg"].astype(np.float64)
        zg1 = results[ii * NK + 1]["out_zg"].astype(np.float64)
        z = zg0[:, :ITILES] + zg1[:, :ITILES]
        g = zg0[:, ITILES:] + zg1[:, ITILES:]
        total += (g / z).sum()
    # term4: per K shard, Z'/N summed over the 4 I cores, then sum_k N/Z'.
    for kk in range(NK):
        acc = np.zeros((2 * KSH,), np.float64)
        for ii in range(NI):
            acc += results[ii * NK + kk]["out_nz"][0].astype(np.float64)
        total += (acc[KSH:] / acc[:KSH]).sum()
    return total + float(np.asarray(ffn_b)[0])


def kernel(u_p, u_c, w_a, ffn_w, ffn_b, **run_kwargs):
    nc = _get_nc()
    maps = _in_maps(
        np.asarray(u_p, np.float32),
        np.asarray(u_c, np.float32),
        np.asarray(w_a, np.float32),
        np.asarray(ffn_w, np.float32),
    )
    res = run_bass_kernel_spmd(nc, maps, core_ids=list(range(NCORES)), **run_kwargs)
    out = np.array([max(_reduce(res.results, ffn_b), 0.0)], dtype=np.float32)
    if run_kwargs:
        return out, res
    return out
